# revision 37
# baseline (speedup 1.0000x reference)
"""Multi-head attention (B=4, S=2048, D=1024, H=16) on 8 Trainium2 cores.

Sharding: data-parallel over the 4 batches x tensor-parallel over 2 groups
of 8 heads. Core c handles batch c//2, head group c%2. Each core computes
its group's slice of the out-projection; the host sums the two partial
outputs per batch.

All matmul operands are bf16 (fp32 PSUM accumulation); rel-err budget is
2e-2 so bf16 rounding (~0.4%) is fine and it halves both PE streaming and
DMA cost vs float32r.

Device-side layout (per core):
  qhT/khT [128, 4, S] bf16 : projections transposed (head-pair dim on
                     partitions: head 2j at partitions 0-63, head 2j+1 at
                     64-127; sequence on free dim).
  scores  : per head pair, key tiles on partitions, 512-query chunks; the
            two heads of a pair issue as back-to-back matmuls on row groups
            (0,0)/(64,0) so they run concurrently in the PE array.
  softmax : exp on ScalarE straight out of PSUM in [128, 2, 1, 512] groups
            (1024 elem/instr); denominators from a ones column appended to
            V during the attn@V accumulation.
  outT [8, 4, 128, 512] f32 : transposed partial out-projection, summed on
            host.
"""
import sys

for _p in ("/opt/trn_rl_repo", "/root/.axon_site/_ro/trn_rl_repo"):
    if _p not in sys.path:
        sys.path.append(_p)

import numpy as np
from ml_dtypes import bfloat16

import concourse.bass as bass
import concourse.tile as tile
from concourse import bacc, mybir
from concourse.bass_utils import run_bass_kernel_spmd

N_CORES = 8
B, S, DIM, H, DK = 4, 2048, 1024, 16, 64
JG = DIM // 2          # head-group width (8 heads x 64)
HPG = 8                # heads per group
BF16 = mybir.dt.bfloat16
F32 = mybir.dt.float32

N_KC = DIM // 128      # contraction chunks for projections
N_JT = JG // 128       # 128-row tiles of the group width
N_SJT = S // 128       # key tiles
QC = 512               # queries per attention chunk
N_QC = S // QC         # attention chunks
SJ_GRP = 1             # key tiles per score/exp group (2 PSUM banks)


ET_SPLIT = 4           # et tiles per pair (quarters of the sjt range)
ET_CH = N_SJT // ET_SPLIT


class EtParts:
    """et for one head pair, split into ET_SPLIT tiles along sjt so buffer
    slots recycle at sub-pair granularity (prev-2 attn@V pipeline)."""

    def __init__(self, parts):
        self.parts = parts

    def sjt(self, h2, j):
        return self.parts[j // ET_CH][:, h2, j % ET_CH, :]

    def grp(self, g0, g1):
        assert g1 - g0 == 1
        return self.parts[g0 // ET_CH][:, :, g0 % ET_CH:g0 % ET_CH + 1, :]


def build_program(phases="ABC", unpack_probe=False):
    nc = bacc.Bacc("TRN2", target_bir_lowering=False, debug=False,
                   num_devices=N_CORES)
    # x inputs pre-chunked [sh, sc, p, kc, 512]: each (sh, sc) half-tile is
    # contiguous per partition (8KB runs) for full-rate DMA
    xqT = nc.dram_tensor("xqT", [2, 2, 128, N_KC, 512], BF16,
                         kind="ExternalInput").ap()
    xkT = nc.dram_tensor("xkT", [2, 2, 128, N_KC, 512], BF16,
                         kind="ExternalInput").ap()
    xvT = nc.dram_tensor("xvT", [2, 2, 128, N_KC, 512], BF16,
                         kind="ExternalInput").ap()
    wqT = nc.dram_tensor("wqT", [128, N_KC, JG], BF16,
                         kind="ExternalInput").ap()
    wkT = nc.dram_tensor("wkT", [128, N_KC, JG], BF16,
                         kind="ExternalInput").ap()
    wvT = nc.dram_tensor("wvT", [128, N_KC, JG], BF16,
                         kind="ExternalInput").ap()
    woT = nc.dram_tensor("woT", [128, N_JT, DIM], BF16,
                         kind="ExternalInput").ap()
    bq = nc.dram_tensor("bq", [128, N_JT], F32, kind="ExternalInput").ap()
    bk = nc.dram_tensor("bk", [128, N_JT], F32, kind="ExternalInput").ap()
    bvr = nc.dram_tensor("bvr", [128, JG], F32, kind="ExternalInput").ap()
    outT = nc.dram_tensor("outT", [DIM // 128, N_QC, 128, QC], F32,
                          kind="ExternalOutput").ap()

    with tile.TileContext(nc) as tc:
        with (
            tc.tile_pool(name="wproj", bufs=3) as wpool,
            tc.tile_pool(name="wo", bufs=1) as wopool,
            tc.tile_pool(name="xin", bufs=5) as xpool,
            tc.tile_pool(name="bias", bufs=1) as bpool,
            tc.tile_pool(name="qk", bufs=1) as qkpool,
            tc.tile_pool(name="vp", bufs=1) as vpool,
            tc.tile_pool(name="attn", bufs=2) as apool,
            tc.tile_pool(name="exp", bufs=8) as epool,
            tc.tile_pool(name="small", bufs=1) as spool,
            tc.tile_pool(name="outsb", bufs=2) as opool,
        ):
            # ---- persistent SBUF residents ----
            qhT = qkpool.tile([128, N_JT, S], BF16, tag="qhT")
            khT = qkpool.tile([128, N_JT, S], BF16, tag="khT")
            v_sb = vpool.tile([128, N_SJT, HPG, DK + 1], BF16, tag="v")
            wo_sb = wopool.tile([128, N_JT, DIM], BF16, tag="wo")
            bqk_sb = bpool.tile([128, 2, N_JT], F32, tag="bqk")
            bq_sb = bqk_sb[:, 0, :]
            bk_sb = bqk_sb[:, 1, :]
            bvr_sb = bpool.tile([128, JG], F32, tag="bvr")

            wk_sb = wpool.tile([128, N_KC, JG], BF16, tag="w", name="wk_sb")
            wq_sb = wpool.tile([128, N_KC, JG], BF16, tag="w", name="wq_sb")
            wv_sb = wpool.tile([128, N_KC, JG], BF16, tag="w", name="wv_sb")
            # wk split per contraction chunk: the first k matmul only needs
            # chunk 0, so it unblocks early
            for _kc in range(N_KC):
                nc.scalar.dma_start(wk_sb[:, _kc, :], wkT[:, _kc, :])
            nc.scalar.dma_start(wq_sb[:], wqT[:])
            nc.scalar.dma_start(wv_sb[:], wvT[:])
            nc.sync.dma_start(bqk_sb[:, 0, :], bq[:])
            nc.sync.dma_start(bqk_sb[:, 1, :], bk[:])
            nc.sync.dma_start(bvr_sb[:], bvr[:])
            # ones column for the softmax denominators
            nc.vector.memset(v_sb[:, :, :, DK:DK + 1], 1.0)
            # touch Exp early so the ACT table set loads during phase A
            # (rides in the den slot, overwritten before first real use)
            warm = spool.tile([1, 2, QC], F32, tag="den", name="warm")
            nc.vector.memset(warm[:, 0, 0:2], 0.0)
            nc.scalar.activation(warm[:, 0, 0:2], warm[:, 0, 0:2],
                                 mybir.ActivationFunctionType.Exp)

            # ---- phase A head: K projection + Q first half ----
            # V projection and Q second half are deferred into the early
            # attention windows as filler work, so ScalarE starts exp ~40us
            # earlier. x loads batched: one 2MB DMA per (input, seq-half) so
            # the ~2us per-DMA completion latency amortizes over 8 kc chunks.
            fillers = []
            if "A" in phases:
             with tc.tile_pool(name="psA", bufs=8, space="PSUM") as psA:
                # head: full K projection (consuming [sh, sc] half-tiles as
                # they arrive), then just the Q slice the first two windows
                # need (head pairs 0-1, queries 0-511)
                xk = {}
                for sh in range(2):
                    for sc in range(2):
                        xt = xpool.tile([128, N_KC, 512], BF16, tag="x",
                                        name=f"xk{sh}{sc}")
                        if sh == 0 and sc == 0:
                            # split the very first load so the kc=0 matmuls
                            # start after 128KB instead of the full 1MB
                            nc.sync.dma_start(xt[:, 0, :], xkT[0, 0, :, 0, :])
                            nc.sync.dma_start(xt[:, 1:, :],
                                              xkT[0, 0, :, 1:, :])
                        else:
                            nc.sync.dma_start(xt[:], xkT[sh, sc])
                        xk[sh, sc] = xt
                xq00 = xpool.tile([128, N_KC, 512], BF16, tag="x",
                                  name="xq00")
                nc.sync.dma_start(xq00[:], xqT[0, 0])
                for sh in range(2):
                    for sc in range(2):
                        psk = [psA.tile([128, 512], F32, tag="ps",
                                        name=f"psk_{i}") for i in range(4)]
                        for kc in range(N_KC):
                            for jt in range(N_JT):
                                nc.tensor.matmul(
                                    psk[jt][:],
                                    wk_sb[:, kc, jt * 128:(jt + 1) * 128],
                                    xk[sh, sc][:, kc, :],
                                    start=(kc == 0), stop=(kc == N_KC - 1))
                        o0 = sh * 1024 + sc * 512
                        for jt in range(N_JT):
                            nc.vector.tensor_scalar_add(
                                khT[:, jt, o0:o0 + 512], psk[jt][:],
                                bk_sb[:, jt:jt + 1])
                psq2 = [psA.tile([128, 512], F32, tag="ps",
                                 name=f"psq_{i}") for i in range(2)]
                for kc in range(N_KC):
                    for jt in range(2):
                        nc.tensor.matmul(
                            psq2[jt][:],
                            wq_sb[:, kc, jt * 128:(jt + 1) * 128],
                            xq00[:, kc, :],
                            start=(kc == 0), stop=(kc == N_KC - 1))
                for jt in range(2):
                    nc.vector.tensor_scalar_add(
                        qhT[:, jt, 0:512], psq2[jt][:],
                        bq_sb[:, jt:jt + 1])

                # deferred inputs: V (all 4 half-tiles) and the rest of Q
                # are projected inside the early attention windows (fillers)
                xv = {}
                for sh in range(2):
                    for sc in range(2):
                        xt = xpool.tile([128, N_KC, 512], BF16, tag="x",
                                        name=f"xv{sh}{sc}")
                        nc.sync.dma_start(xt[:], xvT[sh, sc])
                        xv[sh, sc] = xt
                xq = {(0, 0): xq00}
                for sh, sc in ((0, 1), (1, 0), (1, 1)):
                    xt = xpool.tile([128, N_KC, 512], BF16, tag="x",
                                    name=f"xq{sh}{sc}")
                    nc.sync.dma_start(xt[:], xqT[sh, sc])
                    xq[sh, sc] = xt

                def v_thunk(st, pool):
                    sh, st8 = st // 8, st % 8
                    sc, c0 = st8 // 4, (st8 % 4) * 128
                    psv = pool.tile([128, 512], F32, tag="pj", name="pj")
                    for kc in range(N_KC):
                        nc.tensor.matmul(
                            psv[:],
                            xv[sh, sc][:, kc, c0:c0 + 128],
                            wv_sb[:, kc, :],
                            start=(kc == 0), stop=(kc == N_KC - 1))
                    nc.vector.tensor_tensor(
                        v_sb[:, st, :, 0:DK],
                        psv[:].rearrange("p (h d) -> p h d", h=HPG),
                        bvr_sb[:, :].rearrange("p (h d) -> p h d", h=HPG),
                        mybir.AluOpType.add)

                def q_thunk(jt, sh, sc, pool):
                    psq = pool.tile([128, 512], F32, tag="pj", name="pj")
                    for kc in range(N_KC):
                        nc.tensor.matmul(
                            psq[:],
                            wq_sb[:, kc, jt * 128:(jt + 1) * 128],
                            xq[sh, sc][:, kc, :],
                            start=(kc == 0), stop=(kc == N_KC - 1))
                    nc.vector.tensor_scalar_add(
                        qhT[:, jt,
                            sh * 1024 + sc * 512:sh * 1024 + (sc + 1) * 512],
                        psq[:], bq_sb[:, jt:jt + 1])

                # order by deadline: Q for windows (0,2)/(0,3), then V in st
                # order (the first deferred attn@V drains it sjt-major from
                # window (0,2)), then the remaining Q slices
                for jt in (2, 3):
                    fillers.append(("q", jt, 0, 0))
                for st in range(N_SJT):
                    fillers.append(("v", st))
                for jt in range(N_JT):
                    fillers.append(("q", jt, 0, 1))
                for sc in range(2):
                    for jt in range(N_JT):
                        fillers.append(("q", jt, 1, sc))

            # wo is first needed in phase C — load after projection weights
            nc.scalar.dma_start(wo_sb[:], woT[:])

            # ---- phases B/C: attention + out-projection, per query chunk ----
            # Pair pipeline: window (qc, hp) computes scores+exp for head
            # pair hp (the two heads issue as back-to-back matmuls on row
            # groups 0-63/64-127, running concurrently in the PE array) and
            # interleaves the DEFERRED attn@V of the previous pair (whose
            # et is fully staged in SBUF). Deferring attn@V lets the A/B
            # accumulations share one PSUM bank sequentially.
            if "B" in phases:
             with (
                tc.tile_pool(name="psS", bufs=2, space="PSUM") as psS,
                tc.tile_pool(name="psPA", bufs=1, space="PSUM") as psPA,
                tc.tile_pool(name="psPO", bufs=2, space="PSUM") as psPO,
            ):
                grp = list(range(0, N_SJT, SJ_GRP)) + [N_SJT]
                n_grp = len(grp) - 1

                def run_filler(f):
                    if f[0] == "v":
                        v_thunk(f[1], psPO)
                    else:
                        q_thunk(f[1], f[2], f[3], psPO)

                def outproj(at_tile, qc_idx, ct, pool=None):
                    # psPO doubles as filler scratch; the flush alternates
                    # finish) so consecutive out-projections don't serialize
                    # on the PSUM->SBUF copy
                    if pool is None or pool is psPO:
                        po = psPO.tile([128, 512], F32, tag="pj",
                                       name="pj")[:, :QC]
                    else:
                        po = pool.tile([128, 2, QC], F32, tag="pa",
                                       name="pa")[:, 0, :]
                    for jc in range(N_JT):
                        nc.tensor.matmul(
                            po[:],
                            wo_sb[:, jc, ct * 128:(ct + 1) * 128],
                            at_tile[:, jc, :],
                            start=(jc == 0), stop=(jc == N_JT - 1))
                    ob = opool.tile([128, QC], F32, tag="ob", name="ob")
                    nc.vector.tensor_copy(ob[:], po[:])
                    nc.sync.dma_start(outT[ct, qc_idx], ob[:])

                def issue_attnv(pv, k0, k1):
                    # deferred attn@V matmuls k0..k1 of pair pv, sjt-major
                    # (both heads of tile sjt back-to-back) so et quarters
                    # drain in allocation order for just-in-time slot reuse
                    for kk in range(k0, k1):
                        sjt, h2 = kk // 2, kk % 2
                        nc.tensor.matmul(
                            pv["pah"](h2),
                            v_sb[:, sjt, 2 * pv["hp"] + h2, :],
                            pv["et"].sjt(h2, sjt),
                            start=(sjt == 0), stop=(sjt == N_SJT - 1))

                def finish_pair(pv):
                    hp = pv["hp"]
                    den = spool.tile([1, 2, QC], F32, tag="den")
                    bc = spool.tile([DK, 2, QC], F32, tag="bc")
                    for h2 in range(2):
                        pah = pv["pah"](h2)
                        nc.vector.reciprocal(den[:, h2], pah[DK:DK + 1, :])
                        nc.gpsimd.partition_broadcast(bc[:, h2], den[:, h2])
                        nc.vector.tensor_tensor(
                            pv["at"][h2 * 64:h2 * 64 + 64, hp, :],
                            pah[:DK, :], bc[:, h2],
                            mybir.AluOpType.mult)

                at_tiles = {}
                pending = []
                pipeline = []   # pairs awaiting their deferred attn@V
                # filler pop slots per window, derived from projection
                # deadlines (V by the window-2 attn@V ramp; Q slice (jt, sh,
                # sc) by window 4*(2*sh+sc) group 0 for pair jt)
                POPS = {0: (0, 2, 4, 6, 8, 10, 12, 14),
                        1: (0, 2, 4, 6, 8, 10, 12, 14),
                        2: (1, 3, 9, 15), 3: (5, 11),
                        4: (3, 9, 15), 5: (5, 11), 6: (5, 11), 7: (5,)}
                for qc in range(N_QC):
                    si = slice(qc * QC, (qc + 1) * QC)
                    at_sb = apool.tile([128, N_JT, QC], BF16, tag="at")
                    at_tiles[qc] = at_sb
                    for hp in range(N_JT):
                        w = 4 * qc + hp   # window index
                        # et as ET_SPLIT tiles per pair in an 8-slot ring:
                        # pair w's quarter j lands in the slot pair w-2's
                        # quarter j just drained (attn@V is front-loaded to
                        # guarantee the drain happens first)
                        et = EtParts([
                            epool.tile([128, 2, ET_CH, QC], BF16,
                                       tag="e", name=f"et_{j}")
                            for j in range(ET_SPLIT)])
                        # last pair accumulates per-head in the two psPO
                        # slots (fillers are long done) and its attn@V is
                        # issued just-in-time inside this window, quarter by
                        # quarter behind the exp, so almost nothing is left
                        # for the flush
                        last = qc == N_QC - 1 and hp == N_JT - 1
                        if not last:
                            pa = psPA.tile([128, 2, QC], F32, tag="pa",
                                           name="pa")[:DK + 1]
                            pah = (lambda h2, pa=pa: pa[:, h2, :])
                        else:
                            pa15 = [psPO.tile([128, QC], F32, tag="pj",
                                              name=f"pa15_{h}")[:DK + 1]
                                    for h in range(2)]
                            pah = (lambda h2, pa15=pa15: pa15[h2])
                        # window w runs the attn@V of pair w-2 (2-deep
                        # defer): relaxes the V-projection deadline to
                        # window 2 and lets exp start without waiting on
                        # the preceding pair's attn@V drain
                        me = {"hp": hp, "et": et, "pah": pah, "at": at_sb}
                        # windows 2-4 drain the pair from 2 windows back
                        # (prev-2: V is still being projected); window 5
                        # additionally drains pair 4 in its tail, and from
                        # window 6 on the pipeline runs at depth 1 (prev-1)
                        # so no window ever carries more than ~1 attn@V set
                        if len(pipeline) >= 2 or (pipeline and w >= 6):
                            cur = pipeline[0]
                        else:
                            cur = None
                        nmm_prev = 2 * N_SJT if cur is not None else 0
                        done = 0
                        cur2 = pipeline[1] if w == 5 and \
                            len(pipeline) >= 2 else None
                        done2 = 0
                        done3 = 0
                        for gi in range(n_grp):
                            g0, g1 = grp[gi], grp[gi + 1]
                            gn = g1 - g0
                            ps = psS.tile([128, 2, SJ_GRP, QC], F32,
                                          tag="ps")
                            # row range for the second head of the pair:
                            # 64:128 normally (concurrent row groups);
                            # unpack_probe forces 0:64 (serialized, timing
                            # probe only — results wrong for head B)
                            pb = 0 if unpack_probe else 64
                            for i in range(gn):
                                sjt = g0 + i
                                ks = slice(sjt * 128, (sjt + 1) * 128)
                                nc.tensor.matmul(
                                    ps[:, 0, i, :], khT[0:64, hp, ks],
                                    qhT[0:64, hp, si],
                                    start=True, stop=True)
                                nc.tensor.matmul(
                                    ps[:, 1, i, :], khT[pb:pb + 64, hp, ks],
                                    qhT[pb:pb + 64, hp, si],
                                    start=True, stop=True)
                            nc.scalar.activation(
                                et.grp(g0, g1), ps[:, :, :gn, :],
                                mybir.ActivationFunctionType.Exp,
                                scale=1.0 / np.sqrt(DK))
                            # leftover projection work rides under the
                            # windows per the deadline-derived schedule
                            if gi in POPS.get(w, ()):
                                if fillers:
                                    run_filler(fillers.pop(0))
                            # deferred attn@V rides under this pair's exp,
                            # front-loaded so et quarter j of pair w-2 is
                            # drained a group before this pair's exp needs
                            # its slot at group 4j (8-slot et ring)
                            tgt = min(nmm_prev, nmm_prev * (gi + 6) // n_grp)
                            if cur is not None and tgt > done:
                                issue_attnv(cur, done, tgt)
                                done = tgt
                                if done == nmm_prev:
                                    # finish as soon as drained: takes the
                                    # reciprocal/normalize off the window
                                    # tail and frees the pa slot early
                                    finish_pair(cur)
                            if cur2 is not None and gi >= 11:
                                t2 = min(2 * N_SJT,
                                         2 * N_SJT * (gi - 10) // 5)
                                if t2 > done2:
                                    issue_attnv(cur2, done2, t2)
                                    done2 = t2
                                    if done2 == 2 * N_SJT:
                                        finish_pair(cur2)
                            if last and gi % 4 == 3:
                                # this pair's own attn@V, one quarter behind
                                # the exp
                                issue_attnv(me, done3, done3 + 8)
                                done3 += 8
                            if w in (13, 14):
                                opops = (1, 5, 9, 13)
                            elif w == 15:
                                opops = ()
                            else:
                                opops = (4, 10)
                            if "C" in phases and gi in opops and pending:
                                outproj(*pending.pop(0))
                        if cur is not None:
                            if done < nmm_prev:
                                issue_attnv(cur, done, nmm_prev)
                                finish_pair(cur)
                            pipeline.pop(0)
                        if cur2 is not None:
                            if done2 < 2 * N_SJT:
                                issue_attnv(cur2, done2, 2 * N_SJT)
                                finish_pair(cur2)
                            pipeline.pop(0)
                        if last:
                            finish_pair(me)
                        else:
                            pipeline.append(me)
                        if qc > 0 and hp == (0 if qc == 3 else 1) and \
                                "C" in phases:
                            for ct in range(DIM // 128):
                                pending.append((at_tiles[qc - 1], qc - 1, ct))
                # flush: any pairs still awaiting attn@V (none in the
                # standard 16-window run) + remaining out-projections
                for pv in pipeline:
                    issue_attnv(pv, 0, 2 * N_SJT)
                    finish_pair(pv)
                pipeline.clear()
                if "C" in phases:
                    for ct in range(DIM // 128):
                        pending.append((at_tiles[N_QC - 1], N_QC - 1, ct))
                    # dense drain: 4 concurrent accumulators across the
                    # now-idle PSUM banks keep PE back-to-back (no HAM
                    # re-throttle) with copies/DMA overlapped
                    slots = [psS.tile([128, 2, SJ_GRP, QC], F32, tag="ps",
                                      name=f"fpo{i}")[:, 0, 0, :]
                             for i in range(2)]
                    slots.append(psPA.tile([128, 2, QC], F32, tag="pa",
                                           name="fpo2")[:, 0, :])
                    slots.append(psPO.tile([128, QC], F32, tag="pj",
                                           name="fpo3"))
                    for base in range(0, len(pending), 4):
                        batch = pending[base:base + 4]
                        for jc in range(N_JT):
                            for s, (at_t, qci, ct) in enumerate(batch):
                                nc.tensor.matmul(
                                    slots[s],
                                    wo_sb[:, jc, ct * 128:(ct + 1) * 128],
                                    at_t[:, jc, :],
                                    start=(jc == 0), stop=(jc == N_JT - 1))
                        for s, (at_t, qci, ct) in enumerate(batch):
                            ob = opool.tile([128, QC], F32, tag="ob",
                                            name="ob")
                            nc.vector.tensor_copy(ob[:], slots[s])
                            nc.sync.dma_start(outT[ct, qci], ob[:])
                    pending.clear()
            if "B" not in phases:
                with tc.tile_pool(name="fb", bufs=1) as fb:
                    t0 = fb.tile([128, QC], F32)
                    nc.vector.memset(t0[:], 0.0)
                    nc.sync.dma_start(outT[0, 0], t0[:])
    nc.compile()
    return nc


_CACHED_NC = None


def _get_program():
    global _CACHED_NC
    if _CACHED_NC is None:
        _CACHED_NC = build_program()
    return _CACHED_NC


def _make_in_maps(q, k, v, Wq, bq, Wk, bk, Wv, bv, Wo, bo):
    f32 = np.float32

    def chunk_x(x):
        # [S, DIM] -> transposed, pre-chunked [2(sh), 2(sc), 128, N_KC, 512]
        # bf16; each (sh, sc) block is contiguous per partition for fast DMA
        xT = np.asarray(x, f32).T.astype(bfloat16)   # [DIM, S]
        return np.ascontiguousarray(
            xT.reshape(N_KC, 128, 2, 2, 512).transpose(2, 3, 1, 0, 4))

    in_maps = []
    # per-batch transposed activations (shared between the 2 TP cores)
    xT = {}
    for b in range(B):
        xT[b] = (chunk_x(q[b]), chunk_x(k[b]), chunk_x(v[b]))
    wg = {}
    for g in range(2):
        js = slice(g * JG, (g + 1) * JG)

        def tile_w(W):
            # W[js, :].T = [DIM, JG] -> [128, N_KC, JG] bf16
            wT = np.asarray(W, f32)[js, :].T.astype(bfloat16)
            return np.ascontiguousarray(
                wT.reshape(N_KC, 128, JG).transpose(1, 0, 2))

        woT_g = np.asarray(Wo, f32)[:, js].T.astype(bfloat16)   # [JG, DIM]
        wg[g] = {
            "wqT": tile_w(Wq),
            "wkT": tile_w(Wk),
            "wvT": tile_w(Wv),
            "woT": np.ascontiguousarray(
                woT_g.reshape(N_JT, 128, DIM).transpose(1, 0, 2)),
            "bq": np.ascontiguousarray(
                np.asarray(bq, f32)[js].reshape(N_JT, 128).T),
            "bk": np.ascontiguousarray(
                np.asarray(bk, f32)[js].reshape(N_JT, 128).T),
            "bvr": np.ascontiguousarray(
                np.broadcast_to(np.asarray(bv, f32)[js], (128, JG))),
        }
    for c in range(N_CORES):
        b, g = c // 2, c % 2
        m = {"xqT": xT[b][0], "xkT": xT[b][1], "xvT": xT[b][2]}
        m.update(wg[g])
        in_maps.append(m)
    return in_maps


def _gather(results, bo):
    out = np.empty((B, S, DIM), np.float32)
    bo32 = np.asarray(bo, np.float32)
    for b in range(B):
        acc = results[2 * b]["outT"] + results[2 * b + 1]["outT"]
        # [ct, qc, p, s'] -> [DIM, S]
        full = acc.transpose(0, 2, 1, 3).reshape(DIM, S)
        out[b] = full.T + bo32
    return out


def kernel(q, k, v, Wq, bq, Wk, bk, Wv, bv, Wo, bo):
    import time as _time
    nc = _get_program()
    in_maps = _make_in_maps(q, k, v, Wq, bq, Wk, bk, Wv, bv, Wo, bo)
    last_err = None
    for attempt in range(3):
        try:
            res = run_bass_kernel_spmd(nc, in_maps,
                                       core_ids=list(range(N_CORES)))
            return _gather(res.results, bo)
        except Exception as e:  # transient device/tunnel errors
            last_err = e
            _time.sleep(20 * (attempt + 1))
    raise last_err



# revision 70
# speedup vs baseline: 1.5462x; 1.5462x over previous
"""Multi-head attention (B=4, S=2048, D=1024, H=16) on 8 Trainium2 cores.

Sharding: data-parallel over the 4 batches x tensor-parallel over 2 groups
of 8 heads. Core c handles batch c//2, head group c%2. Each core computes
its group's slice of the out-projection; the host sums the two partial
outputs per batch.

All matmul operands are bf16 (fp32 PSUM accumulation); rel-err budget is
2e-2 so bf16 rounding (~0.4%) is fine and it halves both PE streaming and
DMA cost vs float32r.

Device-side layout (per core):
  qhT/khT [128, 4, S] bf16 : projections transposed (head-pair dim on
                     partitions: head 2j at partitions 0-63, head 2j+1 at
                     64-127; sequence on free dim).
  scores  : per head pair, key tiles on partitions, 512-query chunks; the
            two heads of a pair issue as back-to-back matmuls on row groups
            (0,0)/(64,0) so they run concurrently in the PE array.
  softmax : exp on ScalarE straight out of PSUM in [128, 2, 1, 512] groups
            (1024 elem/instr); denominators from a ones column appended to
            V during the attn@V accumulation.
  outT [8, 4, 128, 512] f32 : transposed partial out-projection, summed on
            host.
"""
import sys

for _p in ("/opt/trn_rl_repo", "/root/.axon_site/_ro/trn_rl_repo"):
    if _p not in sys.path:
        sys.path.append(_p)

import numpy as np
from ml_dtypes import bfloat16

import concourse.bass as bass
import concourse.tile as tile
from concourse import bacc, mybir
from concourse.bass_utils import run_bass_kernel_spmd

N_CORES = 8
B, S, DIM, H, DK = 4, 2048, 1024, 16, 64
JG = DIM // 2          # head-group width (8 heads x 64)
HPG = 8                # heads per group
BF16 = mybir.dt.bfloat16
F32 = mybir.dt.float32

N_KC = DIM // 128      # contraction chunks for projections
N_JT = JG // 128       # 128-row tiles of the group width
N_SJT = S // 128       # key tiles
QC = 512               # queries per attention chunk
N_QC = S // QC         # attention chunks
SJ_GRP = 1             # key tiles per score/exp group (2 PSUM banks)


ET_SPLIT = 4           # et tiles per pair (quarters of the sjt range)
ET_CH = N_SJT // ET_SPLIT


class EtParts:
    """et for one head pair, split into ET_SPLIT tiles along sjt so buffer
    slots recycle at sub-pair granularity (prev-2 attn@V pipeline)."""

    def __init__(self, parts):
        self.parts = parts

    def sjt(self, h2, j):
        return self.parts[j // ET_CH][:, h2, j % ET_CH, :]

    def grp(self, g0, g1):
        assert g1 - g0 == 1
        return self.parts[g0 // ET_CH][:, :, g0 % ET_CH:g0 % ET_CH + 1, :]


def build_program(phases="ABC", unpack_probe=False):
    nc = bacc.Bacc("TRN2", target_bir_lowering=False, debug=False,
                   num_devices=N_CORES)
    # x inputs pre-chunked [sh, sc, p, kc, 512]: each (sh, sc) half-tile is
    # contiguous per partition (8KB runs) for full-rate DMA
    xqT = nc.dram_tensor("xqT", [2, 2, 128, N_KC, 512], BF16,
                         kind="ExternalInput").ap()
    xkT = nc.dram_tensor("xkT", [2, 2, 128, N_KC, 512], BF16,
                         kind="ExternalInput").ap()
    xvT = nc.dram_tensor("xvT", [2, 2, 128, N_KC, 512], BF16,
                         kind="ExternalInput").ap()
    # wk and wq interleaved in one tensor: one DMA trigger loads both
    # (the sync-ring sequencer pays ~2.5us per trigger)
    wkqT = nc.dram_tensor("wkqT", [128, N_KC, 2, JG], BF16,
                          kind="ExternalInput").ap()
    wvT = nc.dram_tensor("wvT", [128, N_KC, JG], BF16,
                         kind="ExternalInput").ap()
    woT = nc.dram_tensor("woT", [128, N_JT, DIM], BF16,
                         kind="ExternalInput").ap()
    bqk = nc.dram_tensor("bqk", [128, 2, N_JT], F32,
                         kind="ExternalInput").ap()
    bvr = nc.dram_tensor("bvr", [128, JG], BF16,
                         kind="ExternalInput").ap()
    outT = nc.dram_tensor("outT", [DIM // 128, N_QC, 128, QC], BF16,
                          kind="ExternalOutput").ap()

    with tile.TileContext(nc) as tc:
        with (
            tc.tile_pool(name="wproj", bufs=1) as wpool,
            tc.tile_pool(name="wo", bufs=1) as wopool,
            tc.tile_pool(name="xin", bufs=4) as xpool,
            tc.tile_pool(name="bias", bufs=1) as bpool,
            tc.tile_pool(name="qk", bufs=1) as qkpool,
            tc.tile_pool(name="vp", bufs=1) as vpool,
            tc.tile_pool(name="attn", bufs=2) as apool,
            tc.tile_pool(name="exp", bufs=10) as epool,
            tc.tile_pool(name="small", bufs=1) as spool,
            tc.tile_pool(name="outsb", bufs=2) as opool,
        ):
            # ---- persistent SBUF residents ----
            qhT = qkpool.tile([128, N_JT, S], BF16, tag="qhT")
            khT = qkpool.tile([128, N_JT, S], BF16, tag="khT")
            v_sb = vpool.tile([128, N_SJT, HPG, DK + 1], BF16, tag="v")
            wo_sb = wopool.tile([128, N_JT, DIM], BF16, tag="wo")
            bqk_sb = bpool.tile([128, 2, N_JT], F32, tag="bqk")
            bq_sb = bqk_sb[:, 0, :]
            bk_sb = bqk_sb[:, 1, :]
            bvr_sb = bpool.tile([128, JG], BF16, tag="bvr")

            wkq_sb = wpool.tile([128, N_KC, 2, JG], BF16, tag="wkq",
                                name="wkq_sb")
            wk_sb = wkq_sb[:, :, 0, :]
            wq_sb = wkq_sb[:, :, 1, :]
            wv_sb = wpool.tile([128, N_KC, JG], BF16, tag="w", name="wv_sb")
            # weights ride the sync/pool rings so the ACT queue carries
            # nothing but exp (a DMA trigger occupies the issuing
            # sequencer); wkq/xk00 are split in halves so the first K
            # matmuls start after ~1.5MB instead of 3MB
            nc.sync.dma_start(wkq_sb[:, 0:4], wkqT[:, 0:4])
            nc.gpsimd.dma_start(bqk_sb[:], bqk[:])
            nc.gpsimd.dma_start(bvr_sb[:], bvr[:])
            nc.gpsimd.dma_start(wv_sb[:], wvT[:])
            # ones column for the softmax denominators
            nc.vector.memset(v_sb[:, :, :, DK:DK + 1], 1.0)
            # touch Exp early so the ACT table set loads during phase A
            # (rides in the den slot, overwritten before first real use)
            warm = spool.tile([1, 2, QC], BF16, tag="den", name="warm")
            nc.vector.memset(warm[:, 0, 0:2], 0.0)
            nc.scalar.activation(warm[:, 0, 0:2], warm[:, 0, 0:2],
                                 mybir.ActivationFunctionType.Exp)
            # HAM warm-up: ~4.5us of junk matmuls on v_sb's ones column
            # while the first DMAs land, so the head K projection runs at
            # 2.4GHz instead of the cold 1.2GHz half-clock

            # ---- phase A head: K projection + Q first half ----
            # V projection and Q second half are deferred into the early
            # attention windows as filler work, so ScalarE starts exp ~40us
            # earlier. x loads batched: one 2MB DMA per (input, seq-half) so
            # the ~2us per-DMA completion latency amortizes over 8 kc chunks.
            fillers = []
            if "A" in phases:
             with tc.tile_pool(name="psA", bufs=4, space="PSUM") as psA:
                # head: only what window 0 group 0 needs — K keys 0-511
                # (all pairs) and Q pairs 0-1 queries 0-511. The other
                # three K blocks ride as fillers in window 0's groups.
                # sync-ring order = need order: xk(0,0) (head K), then
                # wq + xq(0,0) (head Q -> first exp), then the K blocks
                # window 0 consumes group by group
                # ~4.5us of junk matmuls (ones column x v_sb garbage) while
                # the first DMAs land, so the head K projection runs at the
                # warm 2.4GHz clock instead of the HAM-cold 1.2GHz
                wmp = psA.tile([128, 512], F32, tag="ps", name="wmp")
                for _i in range(10):
                    nc.tensor.matmul(
                        wmp[0:1, :],
                        v_sb[:, 0, 0, DK:DK + 1],
                        v_sb[:, 0, :, :].rearrange("p h d -> p (h d)")
                            [:, 0:512],
                        start=True, stop=True)
                xk = {}
                xk00 = xpool.tile([128, N_KC, 512], BF16, tag="x",
                                  name="xk00")
                nc.sync.dma_start(xk00[:, 0:4], xkT[0, 0, :, 0:4])
                nc.sync.dma_start(wkq_sb[:, 4:], wkqT[:, 4:])
                nc.sync.dma_start(xk00[:, 4:], xkT[0, 0, :, 4:])
                xk[0, 0] = xk00
                xq00 = xpool.tile([128, N_KC, 512], BF16, tag="x",
                                  name="xq00")
                nc.sync.dma_start(xq00[:], xqT[0, 0])
                for sh, sc in ((0, 1), (1, 0), (1, 1)):
                    xt = xpool.tile([128, N_KC, 512], BF16, tag="x",
                                    name=f"xk{sh}{sc}")
                    nc.sync.dma_start(xt[:], xkT[sh, sc])
                    xk[sh, sc] = xt
                psk = [psA.tile([128, 512], F32, tag="ps",
                                name=f"psk_{i}") for i in range(4)]
                for kc in range(N_KC):
                    for jt in range(N_JT):
                        nc.tensor.matmul(
                            psk[jt][:],
                            wk_sb[:, kc, jt * 128:(jt + 1) * 128],
                            xk[0, 0][:, kc, :],
                            start=(kc == 0), stop=(kc == N_KC - 1))
                for jt in range(N_JT):
                    nc.vector.tensor_scalar_add(
                        khT[:, jt, 0:512], psk[jt][:],
                        bk_sb[:, jt:jt + 1])
                psq2 = [psA.tile([128, 512], F32, tag="ps",
                                 name=f"psq_{i}") for i in range(2)]
                for kc in range(N_KC):
                    for jt in range(2):
                        nc.tensor.matmul(
                            psq2[jt][:],
                            wq_sb[:, kc, jt * 128:(jt + 1) * 128],
                            xq00[:, kc, :],
                            start=(kc == 0), stop=(kc == N_KC - 1))
                for jt in range(2):
                    nc.vector.tensor_scalar_add(
                        qhT[:, jt, 0:512], psq2[jt][:],
                        bq_sb[:, jt:jt + 1])

                # deferred inputs: V (all 4 half-tiles) and the rest of Q
                # are projected inside the early attention windows; their
                # DMA triggers are paced into the window schedule (pool
                # queue) so they don't time-share the DMA engines against
                # the critical xk/xq path during window 0
                xv = {}
                xq = {(0, 0): xq00}

                def x_dma(kind, sh, sc):
                    xt = xpool.tile([128, N_KC, 512], BF16, tag="x",
                                    name=f"x{kind}{sh}{sc}")
                    if kind == "v":
                        nc.gpsimd.dma_start(xt[:], xvT[sh, sc])
                        xv[sh, sc] = xt
                    else:
                        nc.gpsimd.dma_start(xt[:], xqT[sh, sc])
                        xq[sh, sc] = xt

                def v_thunk(st, pool):
                    sh, st8 = st // 8, st % 8
                    sc, c0 = st8 // 4, (st8 % 4) * 128
                    psv = pool.tile([128, 512], F32, tag="pj", name="pj")
                    for kc in range(N_KC):
                        nc.tensor.matmul(
                            psv[:],
                            xv[sh, sc][:, kc, c0:c0 + 128],
                            wv_sb[:, kc, :],
                            start=(kc == 0), stop=(kc == N_KC - 1))
                    with nc.allow_low_precision(
                            reason="bf16 V bias add fits the budget"):
                        nc.vector.tensor_tensor(
                            v_sb[:, st, :, 0:DK],
                            psv[:].rearrange("p (h d) -> p h d", h=HPG),
                            bvr_sb[:, :].rearrange("p (h d) -> p h d",
                                                   h=HPG),
                            mybir.AluOpType.add)

                def q_thunk(jt, sh, sc, pool):
                    psq = pool.tile([128, 512], F32, tag="pj", name="pj")
                    for kc in range(N_KC):
                        nc.tensor.matmul(
                            psq[:],
                            wq_sb[:, kc, jt * 128:(jt + 1) * 128],
                            xq[sh, sc][:, kc, :],
                            start=(kc == 0), stop=(kc == N_KC - 1))
                    nc.vector.tensor_scalar_add(
                        qhT[:, jt,
                            sh * 1024 + sc * 512:sh * 1024 + (sc + 1) * 512],
                        psq[:], bq_sb[:, jt:jt + 1])

                def k_thunk(sh, sc, jh, pool):
                    # K projection for pairs 2*jh, 2*jh+1, keys block
                    # (sh, sc) — the three non-head K blocks ride in
                    # window 0 so scores can start 30us earlier
                    o0 = sh * 1024 + sc * 512
                    for jt in (2 * jh, 2 * jh + 1):
                        psk = pool.tile([128, 512], F32, tag="pj",
                                        name="pj")
                        for kc in range(N_KC):
                            nc.tensor.matmul(
                                psk[:],
                                wk_sb[:, kc, jt * 128:(jt + 1) * 128],
                                xk[sh, sc][:, kc, :],
                                start=(kc == 0), stop=(kc == N_KC - 1))
                        nc.vector.tensor_scalar_add(
                            khT[:, jt, o0:o0 + 512], psk[:],
                            bk_sb[:, jt:jt + 1])

                # order by deadline: the K blocks (window 0), Q for windows
                # (0,2)/(0,3), V in st order (the pair-0 attn@V drains it
                # sjt-major through window 2), then the remaining Q slices
                for sh, sc in ((0, 1), (1, 0), (1, 1)):
                    for jh in range(2):
                        fillers.append(("k", sh, sc, jh))
                for jt in (2, 3):
                    fillers.append(("q", jt, 0, 0))
                for st in range(N_SJT):
                    fillers.append(("v", st))
                for jt in range(N_JT):
                    fillers.append(("q", jt, 0, 1))
                for sc in range(2):
                    for jt in range(N_JT):
                        fillers.append(("q", jt, 1, sc))

            # wo is first needed in phase C — load on the idle pool queue
            nc.gpsimd.dma_start(wo_sb[:], woT[:])

            # ---- phases B/C: attention + out-projection, per query chunk ----
            # Pair pipeline: window (qc, hp) computes scores+exp for head
            # pair hp (the two heads issue as back-to-back matmuls on row
            # groups 0-63/64-127, running concurrently in the PE array) and
            # interleaves the DEFERRED attn@V of the previous pair (whose
            # et is fully staged in SBUF). Deferring attn@V lets the A/B
            # accumulations share one PSUM bank sequentially.
            if "B" in phases:
             with (
                tc.tile_pool(name="psS", bufs=2, space="PSUM") as psS,
                tc.tile_pool(name="psPA", bufs=2, space="PSUM") as psPA,
                tc.tile_pool(name="psPO", bufs=2, space="PSUM") as psPO,
            ):
                grp = list(range(0, N_SJT, SJ_GRP)) + [N_SJT]
                n_grp = len(grp) - 1

                def run_filler(f):
                    if f[0] == "v":
                        v_thunk(f[1], psPO)
                    elif f[0] == "k":
                        k_thunk(f[1], f[2], f[3], psPO)
                    else:
                        q_thunk(f[1], f[2], f[3], psPO)

                def outproj(at_tile, qc_idx, ct, pool=None):
                    # psPO doubles as filler scratch; the flush alternates
                    # finish) so consecutive out-projections don't serialize
                    # on the PSUM->SBUF copy
                    if pool is None or pool is psPO:
                        po = psPO.tile([128, 512], F32, tag="pj",
                                       name="pj")[:, :QC]
                    else:
                        po = pool.tile([128, 2, QC], F32, tag="pa",
                                       name="pa")[:, 0, :]
                    for jc in range(N_JT):
                        nc.tensor.matmul(
                            po[:],
                            wo_sb[:, jc, ct * 128:(ct + 1) * 128],
                            at_tile[:, jc, :],
                            start=(jc == 0), stop=(jc == N_JT - 1))
                    ob = opool.tile([128, QC], BF16, tag="ob", name="ob")
                    nc.vector.tensor_copy(ob[:], po[:])
                    nc.sync.dma_start(outT[ct, qc_idx], ob[:])

                def issue_attnv(pv, k0, k1):
                    # deferred attn@V matmuls k0..k1 of pair pv, sjt-major
                    # (both heads of tile sjt back-to-back) so et quarters
                    # drain in allocation order for just-in-time slot reuse
                    for kk in range(k0, k1):
                        sjt, h2 = kk // 2, kk % 2
                        nc.tensor.matmul(
                            pv["pah"](h2),
                            v_sb[:, sjt, 2 * pv["hp"] + h2, :],
                            pv["et"].sjt(h2, sjt),
                            start=(sjt == 0), stop=(sjt == N_SJT - 1))

                def finish_pair(pv):
                    hp = pv["hp"]
                    den = spool.tile([1, 2, QC], BF16, tag="den")
                    bc = spool.tile([DK, 2, QC], BF16, tag="bc")
                    # bf16 denominators: 0.4%% rounding on a 2e-2 budget
                    ctx = nc.allow_low_precision(
                        reason="bf16 softmax denominators fit the budget")
                    with ctx:
                      for h2 in range(2):
                        pah = pv["pah"](h2)
                        nc.vector.reciprocal(den[:, h2], pah[DK:DK + 1, :])
                        nc.gpsimd.partition_broadcast(bc[:, h2], den[:, h2])
                        nc.vector.tensor_tensor(
                            pv["at"][h2 * 64:h2 * 64 + 64, hp, :],
                            pah[:DK, :], bc[:, h2],
                            mybir.AluOpType.mult)

                at_tiles = {}
                pending = []
                pipeline = []   # pairs awaiting their deferred attn@V
                # filler pop slots per window, derived from projection
                # deadlines: K blocks in window 0 (block (sh,sc) before the
                # scores sweep reaches its keys), V through windows 1-2
                # (paced with the pair-0 attn@V and the et-ring slot
                # deadlines), Q slice (jt, sh, sc) before window
                # 4*(2*sh+sc) reads it
                POPS = {0: (0, 2, 4, 6, 8, 10, 12, 14),
                        1: (0, 2, 4, 6, 8, 10, 12, 14),
                        2: (0, 1, 2, 3, 4, 5, 7, 9, 11, 13),
                        3: (0, 2),
                        4: (3, 9, 15), 5: (5, 11), 6: (5, 11), 7: (5,)}
                # deferred x-tile DMA triggers at (window, group) slots
                DMAS = {(0, 5): ("v", 0, 0), (0, 9): ("v", 0, 1),
                        (0, 13): ("v", 1, 0), (1, 1): ("v", 1, 1),
                        (1, 5): ("q", 0, 1), (1, 9): ("q", 1, 0),
                        (1, 13): ("q", 1, 1)}
                for qc in range(N_QC):
                    si = slice(qc * QC, (qc + 1) * QC)
                    at_sb = apool.tile([128, N_JT, QC], BF16, tag="at")
                    at_tiles[qc] = at_sb
                    for hp in range(N_JT):
                        w = 4 * qc + hp   # window index
                        # et as ET_SPLIT tiles per pair in an 8-slot ring:
                        # pair w's quarter j lands in the slot pair w-2's
                        # quarter j just drained (attn@V is front-loaded to
                        # guarantee the drain happens first)
                        et = EtParts([
                            epool.tile([128, 2, ET_CH, QC], BF16,
                                       tag="e", name=f"et_{j}")
                            for j in range(ET_SPLIT)])
                        # last pair accumulates per-head in the two psPO
                        # slots (fillers are long done) and its attn@V is
                        # issued just-in-time inside this window, quarter by
                        # quarter behind the exp, so almost nothing is left
                        # for the flush
                        last = qc == N_QC - 1 and hp == N_JT - 1
                        if not last:
                            # per-head pa tiles: head A's accumulator ring
                            # advances as soon as head A's finish reads are
                            # done, overlapping the two heads' normalize
                            # chains across windows
                            pa2 = [psPA.tile([128, QC], F32, tag="pa",
                                             name=f"pa_{h}")[:DK + 1]
                                   for h in range(2)]
                            pah = (lambda h2, pa2=pa2: pa2[h2])
                        else:
                            pa15 = [psPO.tile([128, QC], F32, tag="pj",
                                              name=f"pa15_{h}")[:DK + 1]
                                    for h in range(2)]
                            pah = (lambda h2, pa15=pa15: pa15[h2])
                        # window w runs the attn@V of pair w-2 (2-deep
                        # defer): relaxes the V-projection deadline to
                        # window 2 and lets exp start without waiting on
                        # the preceding pair's attn@V drain
                        me = {"hp": hp, "et": et, "pah": pah, "at": at_sb}
                        # windows 2-4 drain the pair from 2 windows back
                        # (prev-2: V is still being projected); window 5
                        # additionally drains pair 4 in its tail, and from
                        # window 6 on the pipeline runs at depth 1 (prev-1)
                        # so no window ever carries more than ~1 attn@V set
                        if len(pipeline) >= 2 or (pipeline and w >= 6):
                            cur = pipeline[0]
                        else:
                            cur = None
                        nmm_prev = 2 * N_SJT if cur is not None else 0
                        done = 0
                        cur2 = pipeline[1] if w == 5 and \
                            len(pipeline) >= 2 else None
                        done2 = 0
                        done3 = 0
                        for gi in range(n_grp):
                            g0, g1 = grp[gi], grp[gi + 1]
                            gn = g1 - g0
                            ps = psS.tile([128, 2, SJ_GRP, QC], F32,
                                          tag="ps")
                            # row range for the second head of the pair:
                            # 64:128 normally (concurrent row groups);
                            # unpack_probe forces 0:64 (serialized, timing
                            # probe only — results wrong for head B)
                            pb = 0 if unpack_probe else 64
                            for i in range(gn):
                                sjt = g0 + i
                                ks = slice(sjt * 128, (sjt + 1) * 128)
                                nc.tensor.matmul(
                                    ps[:, 0, i, :], khT[0:64, hp, ks],
                                    qhT[0:64, hp, si],
                                    start=True, stop=True)
                                nc.tensor.matmul(
                                    ps[:, 1, i, :], khT[pb:pb + 64, hp, ks],
                                    qhT[pb:pb + 64, hp, si],
                                    start=True, stop=True)
                            nc.scalar.activation(
                                et.grp(g0, g1), ps[:, :, :gn, :],
                                mybir.ActivationFunctionType.Exp,
                                scale=1.0 / np.sqrt(DK))
                            # leftover projection work rides under the
                            # windows per the deadline-derived schedule
                            if gi in POPS.get(w, ()):
                                if fillers:
                                    run_filler(fillers.pop(0))
                            if (w, gi) in DMAS:
                                x_dma(*DMAS[w, gi])
            # deferred attn@V rides under this pair's exp. In the
                            # prev-2 zone (w<=5) it is front-loaded so et
                            # quarters drain before the 10-slot ring
                            # re-issues them; pair 0's drain is paced behind
                            # the V-projection fillers; in prev-1 mode the
                            # ring has a full window of slack, so an even
                            # 2-per-group pace avoids flooding the PE queue
                            # ahead of the next scores group
                            if w == 2:
                                tgt = min(nmm_prev, 2 * gi)
                            elif gi < 2:
                                # no attn@V in the first two groups: its
                                # first matmul waits on the previous pair's
                                # finish (pa slot) and would head-of-line
                                # block this window's scores on the in-order
                                # PE queue
                                tgt = 0
                            else:
                                tgt = min(nmm_prev,
                                          nmm_prev * (gi + 6) // n_grp,
                                          done + 6)
                            if cur is not None and tgt > done:
                                issue_attnv(cur, done, tgt)
                                done = tgt
                                if done == nmm_prev:
                                    # finish as soon as drained: takes the
                                    # reciprocal/normalize off the window
                                    # tail and frees the pa slot early
                                    finish_pair(cur)
                            if cur2 is not None and gi >= 11:
                                t2 = min(2 * N_SJT,
                                         2 * N_SJT * (gi - 10) // 5)
                                if t2 > done2:
                                    issue_attnv(cur2, done2, t2)
                                    done2 = t2
                                    if done2 == 2 * N_SJT:
                                        finish_pair(cur2)
                            if last and gi % 4 == 3:
                                # this pair's own attn@V, one quarter behind
                                # the exp
                                issue_attnv(me, done3, done3 + 8)
                                done3 += 8
                            if w in (13, 14):
                                opops = (1, 5, 9, 13)
                            elif w == 15:
                                opops = ()
                            else:
                                opops = (4, 10)
                            if "C" in phases and gi in opops and pending:
                                outproj(*pending.pop(0))
                        if cur is not None:
                            if done < nmm_prev:
                                issue_attnv(cur, done, nmm_prev)
                                finish_pair(cur)
                            pipeline.pop(0)
                        if cur2 is not None:
                            if done2 < 2 * N_SJT:
                                issue_attnv(cur2, done2, 2 * N_SJT)
                                finish_pair(cur2)
                            pipeline.pop(0)
                        if last:
                            finish_pair(me)
                        else:
                            pipeline.append(me)
                        if qc > 0 and hp == (0 if qc == 3 else 1) and \
                                "C" in phases:
                            for ct in range(DIM // 128):
                                pending.append((at_tiles[qc - 1], qc - 1, ct))
                # flush: any pairs still awaiting attn@V (none in the
                # standard 16-window run) + remaining out-projections
                for pv in pipeline:
                    issue_attnv(pv, 0, 2 * N_SJT)
                    finish_pair(pv)
                pipeline.clear()
                if "C" in phases:
                    for ct in range(DIM // 128):
                        pending.append((at_tiles[N_QC - 1], N_QC - 1, ct))
                    # dense drain: 4 concurrent accumulators across the
                    # now-idle PSUM banks keep PE back-to-back (no HAM
                    # re-throttle) with copies/DMA overlapped
                    slots = [psS.tile([128, 2, SJ_GRP, QC], F32, tag="ps",
                                      name=f"fpo{i}")[:, 0, 0, :]
                             for i in range(2)]
                    slots.append(psPA.tile([128, QC], F32, tag="pa",
                                           name="fpo2"))
                    slots.append(psPO.tile([128, QC], F32, tag="pj",
                                           name="fpo3"))
                    for base in range(0, len(pending), 4):
                        batch = pending[base:base + 4]
                        for jc in range(N_JT):
                            for s, (at_t, qci, ct) in enumerate(batch):
                                nc.tensor.matmul(
                                    slots[s],
                                    wo_sb[:, jc, ct * 128:(ct + 1) * 128],
                                    at_t[:, jc, :],
                                    start=(jc == 0), stop=(jc == N_JT - 1))
                        for s, (at_t, qci, ct) in enumerate(batch):
                            ob = opool.tile([128, QC], BF16, tag="ob",
                                            name="ob")
                            nc.vector.tensor_copy(ob[:], slots[s])
                            nc.sync.dma_start(outT[ct, qci], ob[:])
                    pending.clear()
            if "B" not in phases:
                with tc.tile_pool(name="fb", bufs=1) as fb:
                    t0 = fb.tile([128, QC], F32)
                    nc.vector.memset(t0[:], 0.0)
                    nc.sync.dma_start(outT[0, 0], t0[:])
    nc.compile()
    return nc


_CACHED_NC = None


def _get_program():
    global _CACHED_NC
    if _CACHED_NC is None:
        _CACHED_NC = build_program()
    return _CACHED_NC


def _make_in_maps(q, k, v, Wq, bq, Wk, bk, Wv, bv, Wo, bo):
    f32 = np.float32

    def chunk_x(x):
        # [S, DIM] -> transposed, pre-chunked [2(sh), 2(sc), 128, N_KC, 512]
        # bf16; each (sh, sc) block is contiguous per partition for fast DMA
        xT = np.asarray(x, f32).T.astype(bfloat16)   # [DIM, S]
        return np.ascontiguousarray(
            xT.reshape(N_KC, 128, 2, 2, 512).transpose(2, 3, 1, 0, 4))

    in_maps = []
    # per-batch transposed activations (shared between the 2 TP cores)
    xT = {}
    for b in range(B):
        xT[b] = (chunk_x(q[b]), chunk_x(k[b]), chunk_x(v[b]))
    wg = {}
    for g in range(2):
        js = slice(g * JG, (g + 1) * JG)

        def tile_w(W):
            # W[js, :].T = [DIM, JG] -> [128, N_KC, JG] bf16
            wT = np.asarray(W, f32)[js, :].T.astype(bfloat16)
            return np.ascontiguousarray(
                wT.reshape(N_KC, 128, JG).transpose(1, 0, 2))

        woT_g = np.asarray(Wo, f32)[:, js].T.astype(bfloat16)   # [JG, DIM]
        wg[g] = {
            "wkqT": np.ascontiguousarray(
                np.stack([tile_w(Wk), tile_w(Wq)], axis=2)),
            "wvT": tile_w(Wv),
            "woT": np.ascontiguousarray(
                woT_g.reshape(N_JT, 128, DIM).transpose(1, 0, 2)),
            "bqk": np.ascontiguousarray(np.stack(
                [np.asarray(bq, f32)[js].reshape(N_JT, 128).T,
                 np.asarray(bk, f32)[js].reshape(N_JT, 128).T], axis=1)),
            "bvr": np.ascontiguousarray(np.broadcast_to(
                np.asarray(bv, f32)[js].astype(bfloat16), (128, JG))),
        }
    for c in range(N_CORES):
        b, g = c // 2, c % 2
        m = {"xqT": xT[b][0], "xkT": xT[b][1], "xvT": xT[b][2]}
        m.update(wg[g])
        in_maps.append(m)
    return in_maps


def _gather(results, bo):
    out = np.empty((B, S, DIM), np.float32)
    bo32 = np.asarray(bo, np.float32)
    for b in range(B):
        acc = (results[2 * b]["outT"].astype(np.float32)
               + results[2 * b + 1]["outT"].astype(np.float32))
        # [ct, qc, p, s'] -> [DIM, S]
        full = acc.transpose(0, 2, 1, 3).reshape(DIM, S)
        out[b] = full.T + bo32
    return out


def kernel(q, k, v, Wq, bq, Wk, bk, Wv, bv, Wo, bo):
    import time as _time
    nc = _get_program()
    in_maps = _make_in_maps(q, k, v, Wq, bq, Wk, bk, Wv, bv, Wo, bo)
    last_err = None
    for attempt in range(3):
        try:
            res = run_bass_kernel_spmd(nc, in_maps,
                                       core_ids=list(range(N_CORES)))
            return _gather(res.results, bo)
        except Exception as e:  # transient device/tunnel errors
            last_err = e
            _time.sleep(20 * (attempt + 1))
    raise last_err

